# revision 1
# baseline (speedup 1.0000x reference)
"""GNN message-passing kernel for Trainium2, 8 NeuronCores.

Computation (see spec):
  snp [B,S] x filters [F,S] -> gather to nodes via snp_ids [N] -> segment_sum
  by node2gene [N] (sorted) -> mean over F -> MLP(BatchNorm train-mode) -> [B,1]

Algebraic restructure: with fbar[s] = mean_f filters[f,s],
  h1[d,b] = sum_n W1[d, gene(n)] * snp[b, ids[n]] * fbar[ids[n]]
distributes over per-core partial sums (cores shard the snp axis), so only
h1 [64,B] needs an AllReduce.

Per core (snp-range shard, 25000 snps):
  1. load snp slice [128p, 16b, 196j] (s = p*196+j), filters [128p, 8f, 196j]
  2. fsum[p,j] = (1/F) * sum_f filt; snpsc = snp * fsum  (DVE, 128-partition)
  3. stage snpsc to DRAM [16, 25088]; read back replicated into
     table [128 = 16q+b, 25088 s]  (q = gpsimd core group, b = batch)
  4. gpsimd ap_gather per chunk: out[16q+b, i] = table[16q+b, idx_q[i]]
     (each of the 8 Q7 cores gathers its own group's node stream in SBUF;
      s=25000 is a guaranteed-zero column used for run padding)
  5. DVE tensor_reduce over equal-length slot runs -> seg [128, nslot]
     (per group, genes are dealt round-robin by count so all 64 (core,group)
      slot-count profiles align; shared run structure, ~2% padding)
  6. per slab (emitted in data-readiness order across the size-sorted
     chunks, since PE queues are in-order): stream the W1 slab from DRAM,
     PE-transpose seg[:, t*128:(t+1)*128] -> segt[slot, 16q+b], then
  7. PE matmul accumulate: h1 += w1s[:,q,:].T @ segt[:, 16q:16q+16]
  8. AllReduce h1 over 8 cores; BN+relu; W2; BN+relu; [:15]; W3 -> [1,B]

Host does ONLY integer/layout work (slicing, index packing, W1 permutation,
zero padding, identity matrix).
"""

import numpy as np

import concourse.bacc as bacc
import concourse.bass as bass
import concourse.tile as tile
from concourse import mybir, library_config
from concourse.bass_utils import run_bass_kernel_spmd

F32 = mybir.dt.float32
I16 = mybir.dt.int16

B = 16
S = 200000
G = 20000
N = 1000000
F = 8
D = 64
FEAT = 16
MAIN = 15
BN_EPS = 1e-5
CORES = 8
GROUPS = 8

SPC = S // CORES          # 25000 snps per core
PPART = 128
WCH = 196                 # 128 * 196 = 25088 table columns
TR = PPART * WCH          # 25088
ZR = SPC                  # a guaranteed-zero table column (s >= SPC are 0)
MAXCHUNK = 4096           # max gather indices per ap_gather call (Q7 scratch)


# --------------------------------------------------------------------------
# host-side packing (integers / layout only)
# --------------------------------------------------------------------------

def prepare(snp, filters, W1, b1, g1, bb1, W2, b2, g2, bb2, W3, b3,
            snp_ids, node2gene):
    snp = np.asarray(snp, dtype=np.float32)
    filters = np.asarray(filters, dtype=np.float32)
    ids = np.asarray(snp_ids, dtype=np.int64)
    n2g = np.asarray(node2gene, dtype=np.int64)
    w1t = np.ascontiguousarray(np.asarray(W1, dtype=np.float32).T)  # [G, D]

    core_of = (ids // SPC).astype(np.int32)
    s_local = (ids % SPC).astype(np.int32)

    # ---- per (core, group) gene lists ------------------------------------
    per_cg = []
    core_data = []
    for c in range(CORES):
        m = core_of == c
        gsl = s_local[m]
        gg = n2g[m]                     # ascending (n2g sorted globally)
        gstart = np.searchsorted(gg, np.arange(G + 1))
        cnt = np.diff(gstart)
        present = np.nonzero(cnt)[0]
        pcnt = cnt[present]
        # deal genes (sorted by count desc) round-robin into groups so the
        # per-group sorted-count profiles align -> minimal shared-L padding
        order = np.argsort(-pcnt, kind="stable")
        groups = []
        for q in range(GROUPS):
            sel = order[q::GROUPS]
            groups.append((present[sel], pcnt[sel]))
        per_cg.append(groups)
        core_data.append((gsl, gstart))

    # ---- global slot structure: L_j = max count over all (core, group) ---
    nslot = max(len(g[0]) for gs in per_cg for g in gs)
    nslot_pad = -(-nslot // PPART) * PPART
    NSLAB = nslot_pad // PPART
    L = np.ones(nslot_pad, np.int64)
    for gs in per_cg:
        for genes, counts in gs:
            L[:len(counts)] = np.maximum(L[:len(counts)], counts)
    # runs of equal L (L is non-increasing)
    runs = []  # (L, slot0, nslots)
    j = 0
    while j < nslot_pad:
        k = j
        while k < nslot_pad and L[k] == L[j]:
            k += 1
        runs.append((int(L[j]), j, k - j))
        j = k
    off = np.concatenate([[0], np.cumsum(L)])
    ni_used = int(off[-1])

    # ---- chunk the slot sequence into gather calls -----------------------
    pieces = []  # (L, slot0, n)
    for lv, s0, n in runs:
        j = 0
        while j < n:
            take = min(n - j, max(1, MAXCHUNK // lv))
            pieces.append((lv, s0 + j, take))
            j += take
    chunks = []  # list of (pieces, ni_chunk_padded)
    cur, cur_ni = [], 0
    for p in pieces:
        sz = p[0] * p[2]
        if cur and cur_ni + sz > MAXCHUNK:
            chunks.append((cur, -(-cur_ni // 32) * 32))
            cur, cur_ni = [], 0
        cur.append(p)
        cur_ni += sz
    if cur:
        chunks.append((cur, -(-cur_ni // 32) * 32))
    # largest chunk first: the tail after the final gather (last reduces,
    # transposes, matmuls, collective) then trails the smallest chunk
    chunks.sort(key=lambda c: -c[1])
    niw_tot = sum(ni for _, ni in chunks) // 16   # idx free dim (int16 cols)

    # ---- per-core inputs -------------------------------------------------
    in_maps = []
    for c in range(CORES):
        gsl, gstart = core_data[c]
        idx = np.zeros((PPART, niw_tot), np.int16)
        gene_slot = np.full((nslot_pad, GROUPS), -1, np.int64)
        for q in range(GROUPS):
            genes, counts = per_cg[c][q]
            gene_slot[:len(genes), q] = genes
            stream = np.full(ni_used, ZR, np.int32)
            for j in range(len(genes)):
                a, bnd = gstart[genes[j]], gstart[genes[j] + 1]
                stream[off[j]:off[j] + counts[j]] = np.sort(gsl[a:bnd])
            col = 0
            for pcs, ni in chunks:
                sub = np.full(ni, ZR, np.int32)
                pos = 0
                for lv, s0, n in pcs:
                    sub[pos:pos + n * lv] = stream[off[s0]:off[s0] + n * lv]
                    pos += n * lv
                w = sub.reshape(ni // 16, 16).T.astype(np.int16)
                idx[16 * q:16 * q + 16, col:col + ni // 16] = w
                col += ni // 16

        # W1 slot tensor [128, NSLAB, GROUPS, D]
        gs_clip = gene_slot.clip(0)
        w1slot = w1t[gs_clip.reshape(-1)].reshape(nslot_pad, GROUPS, D)
        w1slot *= (gene_slot >= 0)[:, :, None]
        w1p = np.ascontiguousarray(
            w1slot.reshape(NSLAB, PPART, GROUPS, D).transpose(1, 0, 2, 3))

        sl = slice(c * SPC, (c + 1) * SPC)
        snp_l = np.zeros((B, TR), np.float32)
        snp_l[:, :SPC] = snp[:, sl]
        filt_l = np.zeros((F, TR), np.float32)
        filt_l[:, :SPC] = filters[:, sl]

        in_maps.append(dict(
            snp_l=snp_l,
            filt_l=filt_l,
            idx=idx,
            w1p=w1p,
            ident=np.eye(PPART, dtype=np.float32),
            b1=np.asarray(b1, np.float32).reshape(D, 1),
            g1=np.asarray(g1, np.float32).reshape(D, 1),
            bb1=np.asarray(bb1, np.float32).reshape(D, 1),
            w2t=np.ascontiguousarray(np.asarray(W2, np.float32).T),
            b2=np.asarray(b2, np.float32).reshape(FEAT, 1),
            g2=np.asarray(g2, np.float32).reshape(FEAT, 1),
            bb2=np.asarray(bb2, np.float32).reshape(FEAT, 1),
            w3t=np.ascontiguousarray(np.asarray(W3, np.float32).T),
            b3=np.asarray(b3, np.float32).reshape(1, 1),
        ))
    meta = dict(chunks=chunks, nslot_pad=nslot_pad, nslab=NSLAB,
                niw_tot=niw_tot, ni_used=ni_used)
    return in_maps, meta


# --------------------------------------------------------------------------
# device program
# --------------------------------------------------------------------------

def build_program(meta, reps=1):
    chunks = meta["chunks"]
    nslot_pad = meta["nslot_pad"]
    nslab = meta["nslab"]
    niw_tot = meta["niw_tot"]

    nc = bacc.Bacc("TRN2", target_bir_lowering=False, debug=False,
                   num_devices=CORES)

    def din(name, shape, dt=F32):
        return nc.dram_tensor(name, shape, dt, kind="ExternalInput").ap()

    snp_l = din("snp_l", [B, TR])
    filt_l = din("filt_l", [F, TR])
    idx = din("idx", [PPART, niw_tot], I16)
    w1p = din("w1p", [PPART, nslab, GROUPS, D])
    ident = din("ident", [PPART, PPART])
    b1 = din("b1", [D, 1]); g1 = din("g1", [D, 1]); bb1 = din("bb1", [D, 1])
    w2t = din("w2t", [D, FEAT])
    b2 = din("b2", [FEAT, 1]); g2 = din("g2", [FEAT, 1]); bb2 = din("bb2", [FEAT, 1])
    w3t = din("w3t", [MAIN, 1]); b3 = din("b3", [1, 1])
    out = nc.dram_tensor("out", [1, B], F32, kind="ExternalOutput").ap()

    with tile.TileContext(nc) as tc:
      nc.gpsimd.load_library(library_config.ap_gather)
      for _rep in range(reps):
        with (
            tc.tile_pool(name="work", bufs=1) as wp,
            tc.tile_pool(name="segs", bufs=2) as sp,
            tc.tile_pool(name="psum", bufs=1, space="PSUM") as pp,
            tc.tile_pool(name="dram", bufs=1, space="DRAM") as dp,
        ):
            # ---- load constants / indices -------------------------------
            # w1p (5.2MB) is streamed per-slab during the gather window (see
            # below) so it neither competes with the prologue DMAs for bus
            # time nor holds 40KB/partition of SBUF
            idx_sb = wp.tile([PPART, niw_tot], I16, tag="idx")
            id_sb = wp.tile([PPART, PPART], F32, tag="ident")
            nc.scalar.dma_start(out=id_sb[:], in_=ident[:])

            # ---- build scaled table -------------------------------------
            table = wp.tile([PPART, TR], F32, tag="table")
            with tc.tile_pool(name="build", bufs=1) as bp:
                snp_g = bp.tile([PPART, B, WCH], F32, tag="snp")
                filt_g = bp.tile([PPART, F, WCH], F32, tag="filt")
                fsum = bp.tile([PPART, WCH], F32, tag="fsum")
                snpsc = bp.tile([PPART, B, WCH], F32, tag="snpsc")
                stage = dp.tile([B, TR], F32)
                nc.sync.dma_start(
                    out=snp_g[:],
                    in_=snp_l.rearrange("b (p j) -> p b j", p=PPART))
                nc.sync.dma_start(
                    out=filt_g[:],
                    in_=filt_l.rearrange("f (p j) -> p f j", p=PPART))
                nc.sync.dma_start(out=idx_sb[:], in_=idx[:])
                nc.vector.tensor_reduce(
                    out=fsum[:], in_=filt_g[:].rearrange("p f j -> p j f"),
                    axis=mybir.AxisListType.X, op=mybir.AluOpType.add)
                nc.vector.tensor_scalar_mul(
                    out=fsum[:], in0=fsum[:], scalar1=1.0 / F)
                # scale -> stage -> replicate, pipelined in two p-halves
                # (contiguous s ranges) so the replicate DMAs start while
                # the second half still scales
                repl_engines = [nc.sync, nc.scalar, nc.gpsimd]
                HP = PPART // 4
                HS = HP * WCH
                for h in range(4):
                    ps = slice(h * HP, (h + 1) * HP)
                    ss = slice(h * HS, (h + 1) * HS)
                    nc.vector.tensor_tensor(
                        out=snpsc[ps, :, :], in0=snp_g[ps, :, :],
                        in1=fsum[ps, :].unsqueeze(1)
                            .to_broadcast([HP, B, WCH]),
                        op=mybir.AluOpType.mult)
                    nc.gpsimd.dma_start(
                        out=stage[:, ss].rearrange("b (p j) -> p b j", p=HP),
                        in_=snpsc[ps, :, :])
                    for q in range(GROUPS):
                        # last quarter gates the first gather: keep it off
                        # the SWDGE (Pool) queue, whose per-call desc-gen
                        # overhead would land on the table-ready edge
                        engs = repl_engines if h < 3 else repl_engines[:2]
                        eng = engs[(h * GROUPS + q) % len(engs)]
                        eng.dma_start(
                            out=table[16 * q:16 * q + 16, ss],
                            in_=stage[:, ss])

            # gather pool opens only after the build pool's SBUF is freed
            gp_cm = tc.tile_pool(name="gath", bufs=2)
            gp = gp_cm.__enter__()

            # ---- gather + segment reduce --------------------------------
            seg = wp.tile([PPART, nslot_pad], F32, tag="seg")
            col = 0
            for pcs, ni in chunks:
                gt = gp.tile([PPART, MAXCHUNK + 32], F32, tag="gt")
                nc.gpsimd.ap_gather(
                    out_ap=gt[:, 0:ni],
                    in_ap=table[:],
                    idxs_ap=idx_sb[:, col:col + ni // 16],
                    channels=PPART, num_elems=TR, d=1, num_idxs=ni)
                col += ni // 16
                roff = 0
                for lv, s0, n in pcs:
                    src = gt[:, roff:roff + n * lv]
                    if lv == 1:
                        nc.vector.tensor_copy(
                            out=seg[:, s0:s0 + n], in_=src)
                    else:
                        nc.vector.tensor_reduce(
                            out=seg[:, s0:s0 + n],
                            in_=src.rearrange("p (n l) -> p n l", n=n),
                            axis=mybir.AxisListType.X,
                            op=mybir.AluOpType.add)
                    roff += n * lv

            # ---- per-slab: stream W1 slab, transpose, matmul ------------
            # slab t's transpose fires as soon as its seg slots complete
            # (mid-gather); its W1 slab load chases on the Act DGE queue.
            # PE queues are in-order, so emit slabs in the order their seg
            # slots complete across the (size-sorted) gather chunks — not
            # slot order — so no slab blocks an already-complete one.
            done_at = [0] * nslab
            for k, (pcs, _ni) in enumerate(chunks):
                for _lv, s0, n in pcs:
                    for t in range(s0 // PPART,
                                   min(nslab, (s0 + n - 1) // PPART + 1)):
                        done_at[t] = max(done_at[t], k)
            slab_order = sorted(range(nslab), key=lambda t: (done_at[t], t))
            h1_ps = pp.tile([D, B], F32, tag="h1")
            for ti, t in enumerate(slab_order):
                w1s = sp.tile([PPART, GROUPS, D], F32, tag="w1s", bufs=4)
                nc.scalar.dma_start(out=w1s[:], in_=w1p[:, t, :, :])
                tp = pp.tile([PPART, PPART], F32, tag="tp", bufs=2)
                nc.tensor.transpose(
                    out=tp[:], in_=seg[:, t * PPART:(t + 1) * PPART],
                    identity=id_sb[:])
                segt = sp.tile([PPART, PPART], F32, tag="segt", bufs=2)
                nc.vector.tensor_copy(out=segt[:], in_=tp[:])
                for q in range(GROUPS):
                    nc.tensor.matmul(
                        out=h1_ps[:],
                        lhsT=w1s[:, q, :],
                        rhs=segt[:, 16 * q:16 * q + 16],
                        start=(ti == 0 and q == 0),
                        stop=(ti == nslab - 1 and q == GROUPS - 1),
                        skip_group_check=True)

            # ---- all-reduce + MLP tail ----------------------------------
            h1_sb = wp.tile([D, B], F32, tag="h1sb")
            nc.vector.tensor_copy(out=h1_sb[:], in_=h1_ps[:])
            cc_in = dp.tile([D, B], F32)
            cc_out = dp.tile([D, B], F32, addr_space="Shared")
            nc.sync.dma_start(out=cc_in[:], in_=h1_sb[:])
            nc.gpsimd.collective_compute(
                "AllReduce", mybir.AluOpType.add,
                replica_groups=[list(range(CORES))],
                ins=[cc_in.opt()], outs=[cc_out.opt()])
            h1 = wp.tile([D, B], F32, tag="h1r")
            nc.sync.dma_start(out=h1[:], in_=cc_out[:])

            small = [(b1, D), (g1, D), (bb1, D), (b2, FEAT), (g2, FEAT),
                     (bb2, FEAT), (b3, 1)]
            sb = {}
            for ap_, p in small:
                t_ = wp.tile([p, 1], F32, tag=f"sm_{ap_.tensor.name}")
                nc.sync.dma_start(out=t_[:], in_=ap_[:])
                sb[ap_.tensor.name] = t_
            w2t_sb = wp.tile([D, FEAT], F32, tag="w2t")
            nc.sync.dma_start(out=w2t_sb[:], in_=w2t[:])
            w3t_sb = wp.tile([MAIN, 1], F32, tag="w3t")
            nc.sync.dma_start(out=w3t_sb[:], in_=w3t[:])
            eps1 = wp.tile([PPART, 1], F32, tag="eps")
            nc.vector.memset(eps1[:], BN_EPS)

            def batchnorm_relu(x, gamma, beta, p):
                stats = sp.tile([PPART, 6], F32, tag="bnstats")
                mv = sp.tile([PPART, 2], F32, tag="bnmv")
                nc.vector.bn_stats(out=stats[:p, :], in_=x[:])
                nc.vector.bn_aggr(out=mv[:p, :], in_=stats[:p, :])
                inv = sp.tile([PPART, 1], F32, tag="bninv")
                nc.scalar.activation(
                    out=inv[:p, :], in_=mv[:p, 1:2],
                    func=mybir.ActivationFunctionType.Sqrt,
                    bias=eps1[:p, :], scale=1.0)
                nc.vector.reciprocal(out=inv[:p, :], in_=inv[:p, :])
                nc.vector.tensor_scalar(
                    out=x[:], in0=x[:], scalar1=mv[:p, 0:1], scalar2=inv[:p, :],
                    op0=mybir.AluOpType.subtract, op1=mybir.AluOpType.mult)
                nc.vector.tensor_scalar(
                    out=x[:], in0=x[:], scalar1=gamma, scalar2=beta,
                    op0=mybir.AluOpType.mult, op1=mybir.AluOpType.add)
                nc.vector.tensor_relu(out=x[:], in_=x[:])

            nc.vector.tensor_scalar_add(out=h1[:], in0=h1[:], scalar1=sb["b1"][:])
            batchnorm_relu(h1, sb["g1"][:], sb["bb1"][:], D)

            h2_ps = pp.tile([FEAT, B], F32, tag="h2")
            nc.tensor.matmul(out=h2_ps[:], lhsT=w2t_sb[:], rhs=h1[:],
                             start=True, stop=True)
            h2 = wp.tile([FEAT, B], F32, tag="h2sb")
            nc.vector.tensor_copy(out=h2[:], in_=h2_ps[:])
            nc.vector.tensor_scalar_add(out=h2[:], in0=h2[:], scalar1=sb["b2"][:])
            batchnorm_relu(h2, sb["g2"][:], sb["bb2"][:], FEAT)

            h3_ps = pp.tile([1, B], F32, tag="h3")
            nc.tensor.matmul(out=h3_ps[:], lhsT=w3t_sb[:], rhs=h2[:MAIN, :],
                             start=True, stop=True)
            h3 = wp.tile([1, B], F32, tag="h3sb")
            nc.vector.tensor_copy(out=h3[:], in_=h3_ps[:])
            nc.vector.tensor_scalar_add(out=h3[:], in0=h3[:], scalar1=sb["b3"][:])
            nc.sync.dma_start(out=out[:], in_=h3[:])
            gp_cm.__exit__(None, None, None)

    nc.compile()
    return nc


def kernel(**inputs):
    in_maps, meta = prepare(**inputs)
    nc = build_program(meta)
    res = run_bass_kernel_spmd(nc, in_maps, list(range(CORES)))
    logits = res.results[0]["out"]
    return np.ascontiguousarray(logits.T)

